# revision 38
# baseline (speedup 1.0000x reference)
"""Trainium2 Bass kernel for nn_CLRBP_23124103922240.

Math: scores[s, c] = x[s] . W[c] + b[c], softmax over 16 classes, where
W[c] = g * tile4x4(A1[c]) + (1-g) * A2[c],
A1[c] = u1 u1^T - v1 v1^T (64x64, rank 8), A2[c] = u2 u2^T - v2 v2^T
(256x256, rank 2), g = sigmoid(l[0]).

Strategy (dense-W, X-stationary):
  - x is cast to fp16 on host (measured output rel-err 8.0e-3, gate 2e-2)
    and re-laid out per core as xt[p, q, s] = x[s, q*128 + p]: the flat
    65536-pixel axis is split into 512 chunks of 128; the DMA stream is
    16.8 MB/core, half of f32 -- the kernel is DMA-bound at ~360 B/ns.
  - For each chunk q the X block [128 pixels, 128 samples] is the
    *stationary* matmul operand; two [128, 16] moving operands (the g*A1
    table slice and the (1-g)*W2 slice) accumulate scores [128 samples,
    16 classes] directly in PSUM.  No per-sample vector work at all.
  - W1-tiled never materializes: tile4x4 means the moving slice for chunk
    (m, nh) is g*A1[c, m%64, p%64], read from a [128, 64, 16] table
    (p-duplicated so no partition wrap is needed), itself generated on
    device from w1 via a broadcast diag-expansion + one matmul pair.
  - W2 (dense [65536, 16]) is generated on device from the rank-2
    factors: dg = val2 (x) msk2 diag-expansion, then per (32-row block,
    column half) one [32k, 128] x [32k, 512] matmul + PSUM->SBUF copy.
  - All small consts ride in three packed DMAs (cpa/cpb/cpc, ~83 KB,
    ~220 ns) so the serialized DMA-engine stream is >99% input data;
    val2 is expanded on device so cpb stays skinny, and cpa sits exactly
    at the 512 B/partition descriptor cliff below which the cost doubles.
  - Bias enters as a K=1 matmul (ones x b); softmax on [128, 16] f32;
    tapered tail groups shorten the end-of-stream drain.
  - The output store is a PREPARED SWDGE kv_writeback: descriptors are
    generated mid-stream on the idle Pool engine, so after the final
    softmax op the tail pays only trigger + transfer + sem propagation
    instead of HWDGE generation (625 ns) + DGE-to-DMA delay (650 ns).
    Tile's prepare/trigger support has gaps that _patch_writeback_sync /
    _hoist_dmasw_epilogue_wait fix up post-scheduling (see docstrings).

Data-parallel over 8 NeuronCores: batch 1024 -> 128 samples per core.
Timeline (per core): 1.97 us startup (framework start barrier + first
HWDGE+DGE latency), 46.8 us gapless DMA stream at the modeled 360 B/ns,
3.55 us tail (DMA sem props + softmax chain + exit barriers).
"""

import numpy as np

import concourse.bacc as bacc
import concourse.mybir as mybir
import concourse.tile as tile
from concourse.bass_utils import run_bass_kernel_spmd

N_CORES = 8
B, D, C = 1024, 256, 16
BL = B // N_CORES        # 128 samples per core
NQ = (D * D) // 128      # 512 pixel chunks
G = 64                   # chunks per x DMA group
NG = NQ // G             # 32 groups
MBLK = 32                # W2-gen rows per PSUM bank round

F16 = mybir.dt.float16
F32 = mybir.dt.float32
AOP = mybir.AluOpType
AFT = mybir.ActivationFunctionType
AXL = mybir.AxisListType

_cache = {}


def _build():
    if "nc" in _cache:
        return _cache["nc"]

    nc = bacc.Bacc("TRN2", target_bir_lowering=False, debug=False,
                   num_devices=N_CORES)

    xt_d = nc.dram_tensor("xt", [128, NQ, BL], F16, kind="ExternalInput").ap()
    # packed consts: cpa [:, 0:128]=st1, [:,128:192]=val1, [:,192:208]=msk1
    # cpb [0:32, 0:256]=st2, [:,256:272]=msk2, [:,272:274]=+-(1-g) scale
    # (an f32 packed into two f16 slots; val2 = scale * st2 is generated on
    # device -- 256 fewer columns in the serialized DMA stream);
    # cpc [0:1, 0:144]=ones|bias
    cpa_d = nc.dram_tensor("cpa", [128, 256], F16, kind="ExternalInput").ap()
    cpb_d = nc.dram_tensor("cpb", [32, 274], F16, kind="ExternalInput").ap()
    cpc_d = nc.dram_tensor("cpc", [1, 144], F16, kind="ExternalInput").ap()
    # kv_writeback layout [batch=1, dhi=BL, dho=1, n_ctx=C] == [BL, C] flat;
    # the SWDGE descriptors are prepared mid-stream so the end-of-program
    # store skips HWDGE generation + the DGE->DMA delay entirely.
    out_d = nc.dram_tensor("probs", [1, BL, 1, C], F32,
                           kind="ExternalOutput").ap()

    with tile.TileContext(nc) as tc:
        with (
            tc.tile_pool(name="consts", bufs=1) as consts,
            tc.tile_pool(name="xp", bufs=8) as xpool,
            tc.tile_pool(name="fin", bufs=1) as fin,
            tc.tile_pool(name="gps", bufs=2, space="PSUM") as gpspool,
            tc.tile_pool(name="sc", bufs=1, space="PSUM") as scpool,
        ):
            # group schedule: big groups, tapered tail so the end-of-stream
            # drain only covers a couple of chunks (2-chunk floor: a 1-chunk
            # group has 256B descriptors and pays the <512B 2x DMA penalty)
            sizes = [G] * (NQ // G - 1) + [32, 16, 8, 4, 2, 2]
            assert sum(sizes) == NQ
            starts = [sum(sizes[:i]) for i in range(len(sizes))]

            # x DMA stream first so the big transfer starts at t=0; the
            # W2-gen consts (dg, st2) go right behind group 0 so generation
            # starts early
            pre = {}
            xt = xpool.tile([128, sizes[0], BL], F16, tag="xt")
            nc.sync.dma_start(out=xt, in_=xt_d[:, 0:sizes[0], :])
            pre[0] = xt

            cpa = consts.tile([128, 256], F16)
            nc.sync.dma_start(out=cpa, in_=cpa_d)

            xt = xpool.tile([128, sizes[1], BL], F16, tag="xt")
            nc.sync.dma_start(out=xt,
                              in_=xt_d[:, starts[1]:starts[1] + sizes[1], :])
            pre[1] = xt

            cpb = consts.tile([32, 274], F16)
            nc.sync.dma_start(out=cpb, in_=cpb_d)
            cpc = consts.tile([1, 144], F16)
            nc.sync.dma_start(out=cpc, in_=cpc_d)

            # output tile for the prepared writeback (emitted after the
            # divide below; sync fixed up in _patch_writeback_sync)
            probs = fin.tile([128, 1, 1, C], F32, tag="probs")
            st1 = cpa[:, 0:128]
            val1 = cpa[:, 128:192]
            msk1 = cpa[:, 192:208]
            msk2 = cpb[:, 256:272]
            ob = cpc[0:1, :]
            val2 = consts.tile([32, 256], F16, tag="val2")
            nc.vector.tensor_scalar_mul(val2, cpb[:, 0:256],
                                        cpb[:, 272:274].bitcast(F32))

            # ---- A1 table generation (g * A1[c, j, p%64] at [p, j, c]) ----
            # dg1[k, j, c] = val1[k, j] * msk1[k, c], then
            # a1[p, j, c] = sum_k st1[k, p] * dg1[k, j, c]
            dg1 = consts.tile([128, 64, C], F16)
            nc.vector.scalar_tensor_tensor(
                out=dg1,
                in0=val1.rearrange("p (m c) -> p m c", c=1)
                    .broadcast_to([128, 64, C]),
                scalar=1.0,
                in1=msk1.rearrange("p (m c) -> p m c", m=1)
                    .broadcast_to([128, 64, C]),
                op0=AOP.mult, op1=AOP.mult)
            a1 = consts.tile([128, 64, C], F16)
            for h in range(2):
                a1ps = gpspool.tile([128, 32, C], F32)
                nc.tensor.matmul(a1ps, st1, dg1[:, h * 32:(h + 1) * 32, :],
                                 start=True, stop=True)
                dst = a1[:, h * 32:(h + 1) * 32, :]
                if h == 0:
                    nc.scalar.copy(dst, a1ps)
                else:
                    nc.vector.tensor_scalar_add(dst, a1ps, 0.0)

            # ---- W2 dense generation: w2sb[p, nh, m, c] ----
            # dg[k, m, c] = val2[k, m] * msk2[k, c] (expanded per m-block)
            dg = consts.tile([32, D, C], F16)
            w2sb = consts.tile([128, 2, D, C], F16)
            for mblk in range(D // MBLK):
                ms = slice(mblk * MBLK, (mblk + 1) * MBLK)
                nc.vector.scalar_tensor_tensor(
                    out=dg[:, ms, :],
                    in0=val2[:, ms].rearrange("p (m c) -> p m c", c=1)
                        .broadcast_to([32, MBLK, C]),
                    scalar=1.0,
                    in1=msk2.rearrange("p (m c) -> p m c", m=1)
                        .broadcast_to([32, MBLK, C]),
                    op0=AOP.mult, op1=AOP.mult)
                for nh in range(2):
                    gps = gpspool.tile([128, MBLK, C], F32)
                    nc.tensor.matmul(gps, cpb[:, nh * 128:(nh + 1) * 128],
                                     dg[:, ms, :], start=True, stop=True)
                    dst = w2sb[:, nh, ms, :]
                    if nh == 0:
                        nc.scalar.copy(dst, gps)
                    else:
                        nc.vector.tensor_scalar_add(dst, gps, 0.0)

            # ---- main pass: scores accumulate over all 512 chunks ----
            # the bias matmul opens the accumulation so it is off the
            # end-of-stream critical path
            sc = scpool.tile([BL, C], F32)
            nc.tensor.matmul(sc, ob[:, 0:BL], ob[:, BL:BL + C],
                             start=True, stop=False)
            for gi in range(len(sizes)):
                if gi in pre:
                    xt = pre[gi]
                else:
                    xt = xpool.tile([128, sizes[gi], BL], F16, tag="xt")
                    nc.sync.dma_start(
                        out=xt,
                        in_=xt_d[:, starts[gi]:starts[gi] + sizes[gi], :])
                for t in range(sizes[gi]):
                    q = starts[gi] + t
                    m, nh = q // 2, q % 2
                    last = (q == NQ - 1)
                    nc.tensor.matmul(sc, xt[:, t, :], w2sb[:, nh, m, :],
                                     start=False, stop=False)
                    nc.tensor.matmul(sc, xt[:, t, :], a1[:, m % 64, :],
                                     start=False, stop=last)

            # ---- softmax over the 16 free elements ----
            negmax = fin.tile([BL, 1], F32)
            nc.vector.tensor_reduce(out=negmax, in_=sc, axis=AXL.X,
                                    op=AOP.max, negate=True)
            e = fin.tile([BL, C], F32)
            sume = fin.tile([BL, 1], F32)
            nc.scalar.activation(out=e, in_=sc, func=AFT.Exp, bias=negmax,
                                 scale=1.0, accum_out=sume)
            rec = fin.tile([BL, 1], F32)
            nc.vector.reciprocal(rec, sume)
            nc.vector.tensor_scalar_mul(probs[:, 0, 0, :], e, rec)
            # prepared writeback: ctx indices are int32 zeros bitcast out of
            # cpa's zero padding, so the prep's only real deps are the cpa
            # load (early) and the probs RAW edge (relaxed post-scheduling;
            # descriptors only encode the probs ADDRESS -- the data is read
            # when the trigger fires, which is patched to wait for the final
            # DVE-lane count, i.e. the mul above). Desc-gen therefore runs
            # mid-stream on the idle Pool engine and the tail pays only
            # trigger+transfer+sem.
            ctxidx = cpa[:, 208:210].bitcast(mybir.dt.int32)
            dma_sem = nc.alloc_semaphore("probs_dma")
            nc.gpsimd.kv_writeback(out_d, probs, ctxidx,
                                   prepare_only=True, sem=dma_sem)
            nc.gpsimd.trigger_dma(count=None)

    _patch_writeback_sync(nc)
    nc.compile()
    # the end-of-program gather EventSemaphores only exist after compile;
    # BIR serialization for the device happens later (at run time), so this
    # post-compile patch reaches both TimelineSim and hardware
    _hoist_dmasw_epilogue_wait(nc)
    _cache["nc"] = nc
    return nc


def _patch_writeback_sync(nc):
    """Two post-scheduling fixes for the prepared output writeback.

    1. Point the prep's DMA-completion sem (OnUpdate[0], +16) at the DMASW0
       lane semaphore. Tile ticks the DMASW0 lane for a PREPARE_ONLY SWDGE
       prep and makes the epilogue wait on that lane sem, but bass bakes the
       user-passed `sem=` into the descriptor instead -- nothing would ever
       bump the lane sem (deadlock). This restores exactly the wiring Tile
       emits for gen_mode==0 SWDGE DMAs.
    2. Make the trigger wait on the DVE engine-lane sem reaching its final
       count -- the probs-producing divide is the last DVE instruction, so
       this is exactly "probs written" (a dedicated sem can't be used: DVE
       instructions have no spare sync-update slot, walrus rejects a
       then_inc there). The Pool engine-sem wait it replaces (desc-gen
       completion, done mid-stream) is satisfied tens of microseconds
       before the divide, so HW desc-gen-before-trigger ordering is
       preserved with huge margin.
    3. Relax the prep's RAW wait on the divide (a DVE engine-sem wait).
       The prep only encodes the probs ADDRESS into descriptors; the data
       is read when the trigger fires, which fix 2 orders after the
       divide. Leaving the wait in place would serialize the ~1us Q7
       desc-gen into the end-of-program tail.
    """
    prep = None
    trig = None
    dmasw = None
    dve_sem = None
    dve_total = 0

    def walk(blocks):
        nonlocal prep, trig, dmasw, dve_sem, dve_total
        for b in blocks:
            for ins in b.instructions:
                tn = type(ins).__name__
                si = getattr(ins, "sync_info", None)
                if tn == "InstKVWritebackAnt":
                    prep = ins
                elif tn == "InstTriggerDma":
                    trig = ins
                if si is not None:
                    for w in si.on_wait:
                        if w.ant_name and w.ant_name.startswith("DMASW"):
                            dmasw = w
                    for u in si.on_update:
                        if u.ant_name and u.ant_name.startswith("DVE"):
                            dve_sem = u
                            dve_total += 1
                for attr in ("blocks", "body"):
                    sub = getattr(ins, attr, None)
                    if sub:
                        walk(sub)

    walk(nc.m.functions[0].blocks)
    assert prep is not None and trig is not None, (prep, trig)
    assert dmasw is not None, dmasw
    assert dve_sem is not None and dve_total > 0, (dve_sem, dve_total)

    u0 = prep.sync_info.on_update[0]
    assert u0.ant_name == "probs_dma", u0
    u0.id = dmasw.id
    u0.ant_name = dmasw.ant_name

    w0 = trig.sync_info.on_wait[0]
    assert w0.ant_name.startswith("Pool"), w0
    w0.id = dve_sem.id
    w0.ant_name = dve_sem.ant_name
    w0.wait_value = dve_total

    relaxed = 0
    for w in prep.sync_info.on_wait:
        if w.ant_name and w.ant_name.startswith("DVE"):
            w.wait_value = 0
            relaxed += 1
    assert relaxed == 1, [str(w) for w in prep.sync_info.on_wait]


def _hoist_dmasw_epilogue_wait(nc):
    """Move the epilogue's DMASW0 wait into the LAST of the end-of-program
    gather EventSemaphores. These SP-queue instructions each carry a couple
    of lane-completion waits and run back-to-back; every wait must pass
    before the final drain, so their order is semantically free. The DMASW0
    wait (the output writeback, the last thing to finish) sits mid-sequence,
    so the instructions behind it run after the DMA sem instead of hiding
    inside its 900ns propagation. Swapping wait payloads fixes that.
    """
    seen_trigger = False
    gathers = []

    def walk(blocks):
        nonlocal seen_trigger
        for b in blocks:
            for ins in b.instructions:
                if type(ins).__name__ == "InstTriggerDma":
                    seen_trigger = True
                si = getattr(ins, "sync_info", None)
                if (seen_trigger and si and si.on_wait and not si.on_update
                        and type(ins).__name__ == "InstEventSemaphore"
                        and "barrier" not in ins.name):
                    gathers.append(ins)
                for attr in ("blocks", "body"):
                    sub = getattr(ins, attr, None)
                    if sub:
                        walk(sub)

    walk(nc.m.functions[0].blocks)
    holder = None
    hw = None
    for ins in gathers:
        for w in ins.sync_info.on_wait:
            if w.ant_name and w.ant_name.startswith("DMASW"):
                holder, hw = ins, w
    assert holder is not None, [i.name for i in gathers]
    last = gathers[-1]
    if last is holder:
        return
    lw = last.sync_info.on_wait[-1]
    hw.id, lw.id = lw.id, hw.id
    hw.ant_name, lw.ant_name = lw.ant_name, hw.ant_name
    hw.wait_value, lw.wait_value = lw.wait_value, hw.wait_value


def _host_prep(inputs, w1, w2, l, b):
    inputs = np.asarray(inputs, dtype=np.float32)
    w1 = np.asarray(w1, dtype=np.float32)
    w2 = np.asarray(w2, dtype=np.float32)
    l = np.asarray(l, dtype=np.float32)
    b = np.asarray(b, dtype=np.float32)

    g = np.float32(1.0 / (1.0 + np.exp(-np.float32(l[0]))))

    # A1 gen consts: st1[k=(c,r), p] = w1[c, p%64, r];
    # val1[k, j] = sign_r * g * w1[c, j, r]; msk1[k, c'] = (c' == k//8)
    w1t = w1.transpose(0, 2, 1)                                # [c, r, j]
    w1r = w1t.reshape(128, 64)
    st1 = np.concatenate([w1r, w1r], axis=1).astype(np.float16)
    signs = np.array([-1.0] * 4 + [1.0] * 4, np.float32)
    val1 = (w1t * g * signs[None, :, None]).reshape(128, 64)
    val1 = val1.astype(np.float16)
    msk1 = (np.arange(128)[:, None] // 8 ==
            np.arange(C)[None, :]).astype(np.float16)

    # W2 gen consts: st2[k, n] = (u2|v2)[k, n]; msk2[k, c'] = (c' == k%16);
    # scale[k] = +-(1-g) (val2 = scale * st2 is computed on device)
    u2, v2 = w2[:, :, 1], w2[:, :, 0]                          # [16, 256]
    st2 = np.concatenate([u2, v2], axis=0).astype(np.float16)  # [32, 256]
    msk2 = (np.arange(32)[:, None] % 16 ==
            np.arange(C)[None, :]).astype(np.float16)

    cpa = np.zeros((128, 256), np.float16)
    cpa[:, 0:128] = st1
    cpa[:, 128:192] = val1
    cpa[:, 192:208] = msk1
    cpb = np.zeros((32, 274), np.float16)
    cpb[:, 0:256] = st2
    cpb[:, 256:272] = msk2
    scale = np.zeros((32, 1), np.float32)
    scale[0:16] = 1.0 - g
    scale[16:32] = -(1.0 - g)
    cpb[:, 272:274] = scale.view(np.float16)
    cpc = np.zeros((1, 144), np.float16)
    cpc[0, 0:128] = 1.0
    cpc[0, 128:144] = b.astype(np.float16)

    # x: [1024, 256, 256] -> per-core xt[p, q, s] = x[s, q*128 + p]
    xt_all = inputs.astype(np.float16).reshape(N_CORES, BL, NQ, 128)
    xt_all = np.ascontiguousarray(xt_all.transpose(0, 3, 2, 1))

    shared = dict(cpa=cpa, cpb=cpb, cpc=cpc)
    in_maps = []
    for core in range(N_CORES):
        m = dict(shared)
        m["xt"] = xt_all[core]
        in_maps.append(m)
    return in_maps


def kernel(inputs, w1, w2, l, b, _trace=False):
    nc = _build()
    in_maps = _host_prep(inputs, w1, w2, l, b)
    res = run_bass_kernel_spmd(nc, in_maps, core_ids=list(range(N_CORES)),
                               trace=_trace)
    out = np.concatenate([r["probs"].reshape(BL, C) for r in res.results],
                         axis=0)
    if _trace:
        kernel.last_results = res
    return out



# revision 39
# speedup vs baseline: 1.0129x; 1.0129x over previous
"""Trainium2 Bass kernel for nn_CLRBP_23124103922240.

Math: scores[s, c] = x[s] . W[c] + b[c], softmax over 16 classes, where
W[c] = g * tile4x4(A1[c]) + (1-g) * A2[c],
A1[c] = u1 u1^T - v1 v1^T (64x64, rank 8), A2[c] = u2 u2^T - v2 v2^T
(256x256, rank 2), g = sigmoid(l[0]).

Strategy (dense-W, X-stationary):
  - x is cast to fp16 on host (measured output rel-err 8.0e-3, gate 2e-2)
    and re-laid out per core as xt[p, q, s] = x[s, q*128 + p]: the flat
    65536-pixel axis is split into 512 chunks of 128; the DMA stream is
    16.8 MB/core, half of f32 -- the kernel is DMA-bound at ~360 B/ns.
  - For each chunk q the X block [128 pixels, 128 samples] is the
    *stationary* matmul operand; two [128, 16] moving operands (the g*A1
    table slice and the (1-g)*W2 slice) accumulate scores [128 samples,
    16 classes] directly in PSUM.  No per-sample vector work at all.
  - W1-tiled never materializes: tile4x4 means the moving slice for chunk
    (m, nh) is g*A1[c, m%64, p%64], read from a [128, 64, 16] table
    (p-duplicated so no partition wrap is needed), itself generated on
    device from w1 via a broadcast diag-expansion + one matmul pair.
  - W2 (dense [65536, 16]) is generated on device from the rank-2
    factors: dg = val2 (x) msk2 diag-expansion, then per (32-row block,
    column half) one [32k, 128] x [32k, 512] matmul + PSUM->SBUF copy.
  - All small consts ride in three packed DMAs (cpa/cpb/cpc, ~83 KB,
    ~220 ns) so the serialized DMA-engine stream is >99% input data;
    val2 is expanded on device so cpb stays skinny, and cpa sits exactly
    at the 512 B/partition descriptor cliff below which the cost doubles.
  - Bias enters as a K=1 matmul (ones x b); softmax on [128, 16] f32;
    tapered tail groups shorten the end-of-stream drain.
  - The output store is a PREPARED SWDGE kv_writeback: descriptors are
    generated mid-stream on the idle Pool engine, so after the final
    softmax op the tail pays only trigger + transfer + sem propagation
    instead of HWDGE generation (625 ns) + DGE-to-DMA delay (650 ns).
    Tile's prepare/trigger support has gaps that _patch_writeback_sync /
    _hoist_dmasw_epilogue_wait fix up post-scheduling (see docstrings).

Data-parallel over 8 NeuronCores: batch 1024 -> 128 samples per core.
Timeline (per core): 1.97 us startup (framework start barrier + first
HWDGE+DGE latency), 46.8 us gapless DMA stream at the modeled 360 B/ns,
3.55 us tail (DMA sem props + softmax chain + exit barriers).
"""

import numpy as np

import concourse.bacc as bacc
import concourse.mybir as mybir
import concourse.tile as tile
from concourse.bass_utils import run_bass_kernel_spmd

N_CORES = 8
B, D, C = 1024, 256, 16
BL = B // N_CORES        # 128 samples per core
NQ = (D * D) // 128      # 512 pixel chunks
G = 64                   # chunks per x DMA group
NG = NQ // G             # 32 groups
MBLK = 32                # W2-gen rows per PSUM bank round

F16 = mybir.dt.float16
F32 = mybir.dt.float32
AOP = mybir.AluOpType
AFT = mybir.ActivationFunctionType
AXL = mybir.AxisListType

_cache = {}


def _build():
    if "nc" in _cache:
        return _cache["nc"]

    nc = bacc.Bacc("TRN2", target_bir_lowering=False, debug=False,
                   num_devices=N_CORES)

    xt_d = nc.dram_tensor("xt", [128, NQ, BL], F16, kind="ExternalInput").ap()
    # packed consts: cpa [:, 0:128]=st1, [:,128:192]=val1, [:,192:208]=msk1
    # cpb [0:32, 0:256]=st2, [:,256:272]=msk2, [:,272:274]=+-(1-g) scale
    # (an f32 packed into two f16 slots; val2 = scale * st2 is generated on
    # device -- 256 fewer columns in the serialized DMA stream);
    # cpc [0:1, 0:144]=ones|bias
    cpa_d = nc.dram_tensor("cpa", [128, 256], F16, kind="ExternalInput").ap()
    cpb_d = nc.dram_tensor("cpb", [32, 274], F16, kind="ExternalInput").ap()
    cpc_d = nc.dram_tensor("cpc", [1, 144], F16, kind="ExternalInput").ap()
    # kv_writeback layout [batch=1, dhi=BL, dho=1, n_ctx=C] == [BL, C] flat;
    # the SWDGE descriptors are prepared mid-stream so the end-of-program
    # store skips HWDGE generation + the DGE->DMA delay entirely.
    out_d = nc.dram_tensor("probs", [1, BL, 1, C], F32,
                           kind="ExternalOutput").ap()

    with tile.TileContext(nc) as tc:
        with (
            tc.tile_pool(name="consts", bufs=1) as consts,
            tc.tile_pool(name="xp", bufs=8) as xpool,
            tc.tile_pool(name="fin", bufs=1) as fin,
            tc.tile_pool(name="gps", bufs=2, space="PSUM") as gpspool,
            tc.tile_pool(name="sc", bufs=1, space="PSUM") as scpool,
        ):
            # group schedule: big groups, tapered tail so the end-of-stream
            # drain only covers a couple of chunks (2-chunk floor: a 1-chunk
            # group has 256B descriptors and pays the <512B 2x DMA penalty)
            sizes = [G] * (NQ // G - 1) + [32, 16, 8, 4, 2, 2]
            assert sum(sizes) == NQ
            starts = [sum(sizes[:i]) for i in range(len(sizes))]

            # x DMA stream first so the big transfer starts at t=0; the
            # W2-gen consts (dg, st2) go right behind group 0 so generation
            # starts early
            pre = {}
            xt = xpool.tile([128, sizes[0], BL], F16, tag="xt")
            nc.sync.dma_start(out=xt, in_=xt_d[:, 0:sizes[0], :])
            pre[0] = xt

            cpa = consts.tile([128, 256], F16)
            nc.sync.dma_start(out=cpa, in_=cpa_d)

            xt = xpool.tile([128, sizes[1], BL], F16, tag="xt")
            nc.sync.dma_start(out=xt,
                              in_=xt_d[:, starts[1]:starts[1] + sizes[1], :])
            pre[1] = xt

            cpb = consts.tile([32, 274], F16)
            nc.sync.dma_start(out=cpb, in_=cpb_d)
            cpc = consts.tile([1, 144], F16)
            nc.sync.dma_start(out=cpc, in_=cpc_d)

            # output tile for the prepared writeback (emitted after the
            # divide below; sync fixed up in _patch_writeback_sync)
            probs = fin.tile([128, 1, 1, C], F32, tag="probs")
            st1 = cpa[:, 0:128]
            val1 = cpa[:, 128:192]
            msk1 = cpa[:, 192:208]
            msk2 = cpb[:, 256:272]
            ob = cpc[0:1, :]
            val2 = consts.tile([32, 256], F16, tag="val2")
            nc.vector.tensor_scalar_mul(val2, cpb[:, 0:256],
                                        cpb[:, 272:274].bitcast(F32))

            # ---- A1 table generation (g * A1[c, j, p%64] at [p, j, c]) ----
            # dg1[k, j, c] = val1[k, j] * msk1[k, c], then
            # a1[p, j, c] = sum_k st1[k, p] * dg1[k, j, c]
            dg1 = consts.tile([128, 64, C], F16)
            nc.vector.scalar_tensor_tensor(
                out=dg1,
                in0=val1.rearrange("p (m c) -> p m c", c=1)
                    .broadcast_to([128, 64, C]),
                scalar=1.0,
                in1=msk1.rearrange("p (m c) -> p m c", m=1)
                    .broadcast_to([128, 64, C]),
                op0=AOP.mult, op1=AOP.mult)
            a1 = consts.tile([128, 64, C], F16)
            for h in range(2):
                a1ps = gpspool.tile([128, 32, C], F32)
                nc.tensor.matmul(a1ps, st1, dg1[:, h * 32:(h + 1) * 32, :],
                                 start=True, stop=True)
                dst = a1[:, h * 32:(h + 1) * 32, :]
                if h == 0:
                    nc.scalar.copy(dst, a1ps)
                else:
                    nc.vector.tensor_scalar_add(dst, a1ps, 0.0)

            # ---- W2 dense generation: w2sb[p, nh, m, c] ----
            # dg[k, m, c] = val2[k, m] * msk2[k, c] (expanded per m-block)
            dg = consts.tile([32, D, C], F16)
            w2sb = consts.tile([128, 2, D, C], F16)
            for mblk in range(D // MBLK):
                ms = slice(mblk * MBLK, (mblk + 1) * MBLK)
                nc.vector.scalar_tensor_tensor(
                    out=dg[:, ms, :],
                    in0=val2[:, ms].rearrange("p (m c) -> p m c", c=1)
                        .broadcast_to([32, MBLK, C]),
                    scalar=1.0,
                    in1=msk2.rearrange("p (m c) -> p m c", m=1)
                        .broadcast_to([32, MBLK, C]),
                    op0=AOP.mult, op1=AOP.mult)
                for nh in range(2):
                    gps = gpspool.tile([128, MBLK, C], F32)
                    nc.tensor.matmul(gps, cpb[:, nh * 128:(nh + 1) * 128],
                                     dg[:, ms, :], start=True, stop=True)
                    dst = w2sb[:, nh, ms, :]
                    if nh == 0:
                        nc.scalar.copy(dst, gps)
                    else:
                        nc.vector.tensor_scalar_add(dst, gps, 0.0)

            # ---- main pass: scores accumulate over all 512 chunks ----
            # the bias matmul opens the accumulation so it is off the
            # end-of-stream critical path
            sc = scpool.tile([BL, C], F32)
            nc.tensor.matmul(sc, ob[:, 0:BL], ob[:, BL:BL + C],
                             start=True, stop=False)
            for gi in range(len(sizes)):
                if gi in pre:
                    xt = pre[gi]
                else:
                    xt = xpool.tile([128, sizes[gi], BL], F16, tag="xt")
                    nc.sync.dma_start(
                        out=xt,
                        in_=xt_d[:, starts[gi]:starts[gi] + sizes[gi], :])
                for t in range(sizes[gi]):
                    q = starts[gi] + t
                    m, nh = q // 2, q % 2
                    last = (q == NQ - 1)
                    nc.tensor.matmul(sc, xt[:, t, :], w2sb[:, nh, m, :],
                                     start=False, stop=False)
                    nc.tensor.matmul(sc, xt[:, t, :], a1[:, m % 64, :],
                                     start=False, stop=last)

            # ---- softmax over the 16 free elements ----
            negmax = fin.tile([BL, 1], F32)
            nc.vector.tensor_reduce(out=negmax, in_=sc, axis=AXL.X,
                                    op=AOP.max, negate=True)
            e = fin.tile([BL, C], F32)
            sume = fin.tile([BL, 1], F32)
            nc.scalar.activation(out=e, in_=sc, func=AFT.Exp, bias=negmax,
                                 scale=1.0, accum_out=sume)
            rec = fin.tile([BL, 1], F32)
            nc.vector.reciprocal(rec, sume)
            nc.vector.tensor_scalar_mul(probs[:, 0, 0, :], e, rec)
            # prepared writeback: ctx indices are int32 zeros bitcast out of
            # cpa's zero padding, so the prep's only real deps are the cpa
            # load (early) and the probs RAW edge (relaxed post-scheduling;
            # descriptors only encode the probs ADDRESS -- the data is read
            # when the trigger fires, which is patched to wait for the final
            # DVE-lane count, i.e. the mul above). Desc-gen therefore runs
            # mid-stream on the idle Pool engine and the tail pays only
            # trigger+transfer+sem.
            ctxidx = cpa[:, 208:210].bitcast(mybir.dt.int32)
            dma_sem = nc.alloc_semaphore("probs_dma")
            nc.gpsimd.kv_writeback(out_d, probs, ctxidx,
                                   prepare_only=True, sem=dma_sem)
            nc.gpsimd.trigger_dma(count=None)

    _patch_writeback_sync(nc)
    nc.compile()
    # the end-of-program gather EventSemaphores only exist after compile;
    # BIR serialization for the device happens later (at run time), so these
    # post-compile patches reach both TimelineSim and hardware
    _hoist_dmasw_epilogue_wait(nc)
    _hoist_first_dma(nc)
    _cache["nc"] = nc
    return nc


def _hoist_first_dma(nc):
    """Move the first x-group DMA ahead of the SP queue's start-barrier wait.

    The instruction has no semaphore waits, its SBUF target is untouched by
    the preamble, and its consumers all wait on its DMAHW lane semaphore --
    the sync graph is completely unchanged, the DMA simply issues at t~0
    instead of after the ~640 ns all-engine start barrier, so the first
    transfer (and with it the entire gapless stream) starts that much
    earlier. SP arrives at the barrier ~650 ns later, which delays the
    other engines' release by ~120 ns; nothing on those queues is needed
    before the const tables land at ~8 us, so only the stream start moves.
    """
    fn = nc.m.functions[0]
    blocks = list(fn.blocks)
    b0, b1 = blocks[0].instructions, blocks[1].instructions
    idx = None
    for i, ins in enumerate(b1):
        if type(ins).__name__ == "InstDMACopy":
            idx = i
            break
    assert idx is not None
    first = b1[idx]
    si = first.sync_info
    assert not si.on_wait, [str(w) for w in si.on_wait]
    assert si.on_update and si.on_update[0].ant_name.startswith("DMAHW"), (
        [str(u) for u in si.on_update])
    b1.pop(idx)
    b0.insert(1, first)


def _patch_writeback_sync(nc):
    """Two post-scheduling fixes for the prepared output writeback.

    1. Point the prep's DMA-completion sem (OnUpdate[0], +16) at the DMASW0
       lane semaphore. Tile ticks the DMASW0 lane for a PREPARE_ONLY SWDGE
       prep and makes the epilogue wait on that lane sem, but bass bakes the
       user-passed `sem=` into the descriptor instead -- nothing would ever
       bump the lane sem (deadlock). This restores exactly the wiring Tile
       emits for gen_mode==0 SWDGE DMAs.
    2. Make the trigger wait on the DVE engine-lane sem reaching its final
       count -- the probs-producing divide is the last DVE instruction, so
       this is exactly "probs written" (a dedicated sem can't be used: DVE
       instructions have no spare sync-update slot, walrus rejects a
       then_inc there). The Pool engine-sem wait it replaces (desc-gen
       completion, done mid-stream) is satisfied tens of microseconds
       before the divide, so HW desc-gen-before-trigger ordering is
       preserved with huge margin.
    3. Relax the prep's RAW wait on the divide (a DVE engine-sem wait).
       The prep only encodes the probs ADDRESS into descriptors; the data
       is read when the trigger fires, which fix 2 orders after the
       divide. Leaving the wait in place would serialize the ~1us Q7
       desc-gen into the end-of-program tail.
    """
    prep = None
    trig = None
    dmasw = None
    dve_sem = None
    dve_total = 0

    def walk(blocks):
        nonlocal prep, trig, dmasw, dve_sem, dve_total
        for b in blocks:
            for ins in b.instructions:
                tn = type(ins).__name__
                si = getattr(ins, "sync_info", None)
                if tn == "InstKVWritebackAnt":
                    prep = ins
                elif tn == "InstTriggerDma":
                    trig = ins
                if si is not None:
                    for w in si.on_wait:
                        if w.ant_name and w.ant_name.startswith("DMASW"):
                            dmasw = w
                    for u in si.on_update:
                        if u.ant_name and u.ant_name.startswith("DVE"):
                            dve_sem = u
                            dve_total += 1
                for attr in ("blocks", "body"):
                    sub = getattr(ins, attr, None)
                    if sub:
                        walk(sub)

    walk(nc.m.functions[0].blocks)
    assert prep is not None and trig is not None, (prep, trig)
    assert dmasw is not None, dmasw
    assert dve_sem is not None and dve_total > 0, (dve_sem, dve_total)

    u0 = prep.sync_info.on_update[0]
    assert u0.ant_name == "probs_dma", u0
    u0.id = dmasw.id
    u0.ant_name = dmasw.ant_name

    w0 = trig.sync_info.on_wait[0]
    assert w0.ant_name.startswith("Pool"), w0
    w0.id = dve_sem.id
    w0.ant_name = dve_sem.ant_name
    w0.wait_value = dve_total

    relaxed = 0
    for w in prep.sync_info.on_wait:
        if w.ant_name and w.ant_name.startswith("DVE"):
            w.wait_value = 0
            relaxed += 1
    assert relaxed == 1, [str(w) for w in prep.sync_info.on_wait]


def _hoist_dmasw_epilogue_wait(nc):
    """Move the epilogue's DMASW0 wait into the LAST of the end-of-program
    gather EventSemaphores. These SP-queue instructions each carry a couple
    of lane-completion waits and run back-to-back; every wait must pass
    before the final drain, so their order is semantically free. The DMASW0
    wait (the output writeback, the last thing to finish) sits mid-sequence,
    so the instructions behind it run after the DMA sem instead of hiding
    inside its 900ns propagation. Swapping wait payloads fixes that.
    """
    seen_trigger = False
    gathers = []

    def walk(blocks):
        nonlocal seen_trigger
        for b in blocks:
            for ins in b.instructions:
                if type(ins).__name__ == "InstTriggerDma":
                    seen_trigger = True
                si = getattr(ins, "sync_info", None)
                if (seen_trigger and si and si.on_wait and not si.on_update
                        and type(ins).__name__ == "InstEventSemaphore"
                        and "barrier" not in ins.name):
                    gathers.append(ins)
                for attr in ("blocks", "body"):
                    sub = getattr(ins, attr, None)
                    if sub:
                        walk(sub)

    walk(nc.m.functions[0].blocks)
    holder = None
    hw = None
    for ins in gathers:
        for w in ins.sync_info.on_wait:
            if w.ant_name and w.ant_name.startswith("DMASW"):
                holder, hw = ins, w
    assert holder is not None, [i.name for i in gathers]
    last = gathers[-1]
    if last is holder:
        return
    lw = last.sync_info.on_wait[-1]
    hw.id, lw.id = lw.id, hw.id
    hw.ant_name, lw.ant_name = lw.ant_name, hw.ant_name
    hw.wait_value, lw.wait_value = lw.wait_value, hw.wait_value


def _host_prep(inputs, w1, w2, l, b):
    inputs = np.asarray(inputs, dtype=np.float32)
    w1 = np.asarray(w1, dtype=np.float32)
    w2 = np.asarray(w2, dtype=np.float32)
    l = np.asarray(l, dtype=np.float32)
    b = np.asarray(b, dtype=np.float32)

    g = np.float32(1.0 / (1.0 + np.exp(-np.float32(l[0]))))

    # A1 gen consts: st1[k=(c,r), p] = w1[c, p%64, r];
    # val1[k, j] = sign_r * g * w1[c, j, r]; msk1[k, c'] = (c' == k//8)
    w1t = w1.transpose(0, 2, 1)                                # [c, r, j]
    w1r = w1t.reshape(128, 64)
    st1 = np.concatenate([w1r, w1r], axis=1).astype(np.float16)
    signs = np.array([-1.0] * 4 + [1.0] * 4, np.float32)
    val1 = (w1t * g * signs[None, :, None]).reshape(128, 64)
    val1 = val1.astype(np.float16)
    msk1 = (np.arange(128)[:, None] // 8 ==
            np.arange(C)[None, :]).astype(np.float16)

    # W2 gen consts: st2[k, n] = (u2|v2)[k, n]; msk2[k, c'] = (c' == k%16);
    # scale[k] = +-(1-g) (val2 = scale * st2 is computed on device)
    u2, v2 = w2[:, :, 1], w2[:, :, 0]                          # [16, 256]
    st2 = np.concatenate([u2, v2], axis=0).astype(np.float16)  # [32, 256]
    msk2 = (np.arange(32)[:, None] % 16 ==
            np.arange(C)[None, :]).astype(np.float16)

    cpa = np.zeros((128, 256), np.float16)
    cpa[:, 0:128] = st1
    cpa[:, 128:192] = val1
    cpa[:, 192:208] = msk1
    cpb = np.zeros((32, 274), np.float16)
    cpb[:, 0:256] = st2
    cpb[:, 256:272] = msk2
    scale = np.zeros((32, 1), np.float32)
    scale[0:16] = 1.0 - g
    scale[16:32] = -(1.0 - g)
    cpb[:, 272:274] = scale.view(np.float16)
    cpc = np.zeros((1, 144), np.float16)
    cpc[0, 0:128] = 1.0
    cpc[0, 128:144] = b.astype(np.float16)

    # x: [1024, 256, 256] -> per-core xt[p, q, s] = x[s, q*128 + p]
    xt_all = inputs.astype(np.float16).reshape(N_CORES, BL, NQ, 128)
    xt_all = np.ascontiguousarray(xt_all.transpose(0, 3, 2, 1))

    shared = dict(cpa=cpa, cpb=cpb, cpc=cpc)
    in_maps = []
    for core in range(N_CORES):
        m = dict(shared)
        m["xt"] = xt_all[core]
        in_maps.append(m)
    return in_maps


def kernel(inputs, w1, w2, l, b, _trace=False):
    nc = _build()
    in_maps = _host_prep(inputs, w1, w2, l, b)
    res = run_bass_kernel_spmd(nc, in_maps, core_ids=list(range(N_CORES)),
                               trace=_trace)
    out = np.concatenate([r["probs"].reshape(BL, C) for r in res.results],
                         axis=0)
    if _trace:
        kernel.last_results = res
    return out



# revision 42
# speedup vs baseline: 1.0134x; 1.0005x over previous
"""Trainium2 Bass kernel for nn_CLRBP_23124103922240.

Math: scores[s, c] = x[s] . W[c] + b[c], softmax over 16 classes, where
W[c] = g * tile4x4(A1[c]) + (1-g) * A2[c],
A1[c] = u1 u1^T - v1 v1^T (64x64, rank 8), A2[c] = u2 u2^T - v2 v2^T
(256x256, rank 2), g = sigmoid(l[0]).

Strategy (dense-W, X-stationary):
  - x is cast to fp16 on host (measured output rel-err 8.0e-3, gate 2e-2)
    and re-laid out per core as xt[p, q, s] = x[s, q*128 + p]: the flat
    65536-pixel axis is split into 512 chunks of 128; the DMA stream is
    16.8 MB/core, half of f32 -- the kernel is DMA-bound at ~360 B/ns.
  - For each chunk q the X block [128 pixels, 128 samples] is the
    *stationary* matmul operand; two [128, 16] moving operands (the g*A1
    table slice and the (1-g)*W2 slice) accumulate scores [128 samples,
    16 classes] directly in PSUM.  No per-sample vector work at all.
  - W1-tiled never materializes: tile4x4 means the moving slice for chunk
    (m, nh) is g*A1[c, m%64, p%64], read from a [128, 64, 16] table
    (p-duplicated so no partition wrap is needed), itself generated on
    device from w1 via a broadcast diag-expansion + one matmul pair.
  - W2 (dense [65536, 16]) is generated on device from the rank-2
    factors: dg = val2 (x) msk2 diag-expansion, then per (32-row block,
    column half) one [32k, 128] x [32k, 512] matmul + PSUM->SBUF copy.
  - All small consts ride in three packed DMAs (cpa/cpb/cpc, ~83 KB,
    ~220 ns) so the serialized DMA-engine stream is >99% input data;
    val2 is expanded on device so cpb stays skinny, and cpa sits exactly
    at the 512 B/partition descriptor cliff below which the cost doubles.
  - Bias enters as a K=1 matmul (ones x b); softmax on [128, 16] f32;
    tapered tail groups shorten the end-of-stream drain.
  - The output store is a PREPARED SWDGE kv_writeback: descriptors are
    generated mid-stream on the idle Pool engine, so after the final
    softmax op the tail pays only trigger + transfer + sem propagation
    instead of HWDGE generation (625 ns) + DGE-to-DMA delay (650 ns).
    Tile's prepare/trigger support has gaps that _patch_writeback_sync /
    _hoist_dmasw_epilogue_wait fix up post-scheduling (see docstrings).

Data-parallel over 8 NeuronCores: batch 1024 -> 128 samples per core.
Timeline (per core): 1.30 us startup (first-DMA HWDGE+DGE pipeline; the
DMA is hoisted ahead of the start barrier -- see _hoist_first_dma),
46.8 us gapless DMA stream at the modeled 360 B/ns, 3.53 us tail (DMA
sem props + softmax chain + exit barriers).
"""

import numpy as np

import concourse.bacc as bacc
import concourse.mybir as mybir
import concourse.tile as tile
from concourse.bass_utils import run_bass_kernel_spmd

N_CORES = 8
B, D, C = 1024, 256, 16
BL = B // N_CORES        # 128 samples per core
NQ = (D * D) // 128      # 512 pixel chunks
G = 64                   # chunks per x DMA group
NG = NQ // G             # 32 groups
MBLK = 32                # W2-gen rows per PSUM bank round

F16 = mybir.dt.float16
F32 = mybir.dt.float32
AOP = mybir.AluOpType
AFT = mybir.ActivationFunctionType
AXL = mybir.AxisListType

_cache = {}


def _build():
    if "nc" in _cache:
        return _cache["nc"]

    nc = bacc.Bacc("TRN2", target_bir_lowering=False, debug=False,
                   num_devices=N_CORES)

    xt_d = nc.dram_tensor("xt", [128, NQ, BL], F16, kind="ExternalInput").ap()
    # packed consts: cpa [:, 0:128]=st1, [:,128:192]=val1, [:,192:208]=msk1
    # cpb [0:32, 0:256]=st2, [:,256:272]=msk2, [:,272:274]=+-(1-g) scale
    # (an f32 packed into two f16 slots; val2 = scale * st2 is generated on
    # device -- 256 fewer columns in the serialized DMA stream);
    # cpc [0:1, 0:144]=ones|bias
    cpa_d = nc.dram_tensor("cpa", [128, 256], F16, kind="ExternalInput").ap()
    cpb_d = nc.dram_tensor("cpb", [32, 274], F16, kind="ExternalInput").ap()
    cpc_d = nc.dram_tensor("cpc", [1, 144], F16, kind="ExternalInput").ap()
    # kv_writeback layout [batch=1, dhi=BL, dho=1, n_ctx=C] == [BL, C] flat;
    # the SWDGE descriptors are prepared mid-stream so the end-of-program
    # store skips HWDGE generation + the DGE->DMA delay entirely.
    out_d = nc.dram_tensor("probs", [1, BL, 1, C], F32,
                           kind="ExternalOutput").ap()

    with tile.TileContext(nc) as tc:
        with (
            tc.tile_pool(name="consts", bufs=1) as consts,
            tc.tile_pool(name="xp", bufs=8) as xpool,
            tc.tile_pool(name="fin", bufs=1) as fin,
            tc.tile_pool(name="gps", bufs=2, space="PSUM") as gpspool,
            tc.tile_pool(name="sc", bufs=1, space="PSUM") as scpool,
        ):
            # group schedule: big groups, tapered tail so the end-of-stream
            # drain only covers a couple of chunks (2-chunk floor: a 1-chunk
            # group has 256B descriptors and pays the <512B 2x DMA penalty)
            sizes = [G] * (NQ // G - 1) + [32, 16, 8, 4, 2, 2]
            assert sum(sizes) == NQ
            starts = [sum(sizes[:i]) for i in range(len(sizes))]

            # x DMA stream first so the big transfer starts at t=0; the
            # W2-gen consts (dg, st2) go right behind group 0 so generation
            # starts early
            pre = {}
            xt = xpool.tile([128, sizes[0], BL], F16, tag="xt")
            nc.sync.dma_start(out=xt, in_=xt_d[:, 0:sizes[0], :])
            pre[0] = xt

            cpa = consts.tile([128, 256], F16)
            nc.sync.dma_start(out=cpa, in_=cpa_d)

            xt = xpool.tile([128, sizes[1], BL], F16, tag="xt")
            nc.sync.dma_start(out=xt,
                              in_=xt_d[:, starts[1]:starts[1] + sizes[1], :])
            pre[1] = xt

            cpb = consts.tile([32, 274], F16)
            nc.sync.dma_start(out=cpb, in_=cpb_d)
            cpc = consts.tile([1, 144], F16)
            nc.sync.dma_start(out=cpc, in_=cpc_d)

            # output tile for the prepared writeback (emitted after the
            # divide below; sync fixed up in _patch_writeback_sync)
            probs = fin.tile([128, 1, 1, C], F32, tag="probs")
            st1 = cpa[:, 0:128]
            val1 = cpa[:, 128:192]
            msk1 = cpa[:, 192:208]
            msk2 = cpb[:, 256:272]
            ob = cpc[0:1, :]
            val2 = consts.tile([32, 256], F16, tag="val2")
            nc.vector.tensor_scalar_mul(val2, cpb[:, 0:256],
                                        cpb[:, 272:274].bitcast(F32))

            # ---- A1 table generation (g * A1[c, j, p%64] at [p, j, c]) ----
            # dg1[k, j, c] = val1[k, j] * msk1[k, c], then
            # a1[p, j, c] = sum_k st1[k, p] * dg1[k, j, c]
            dg1 = consts.tile([128, 64, C], F16)
            nc.vector.scalar_tensor_tensor(
                out=dg1,
                in0=val1.rearrange("p (m c) -> p m c", c=1)
                    .broadcast_to([128, 64, C]),
                scalar=1.0,
                in1=msk1.rearrange("p (m c) -> p m c", m=1)
                    .broadcast_to([128, 64, C]),
                op0=AOP.mult, op1=AOP.mult)
            a1 = consts.tile([128, 64, C], F16)
            for h in range(2):
                a1ps = gpspool.tile([128, 32, C], F32)
                nc.tensor.matmul(a1ps, st1, dg1[:, h * 32:(h + 1) * 32, :],
                                 start=True, stop=True)
                dst = a1[:, h * 32:(h + 1) * 32, :]
                if h == 0:
                    nc.scalar.copy(dst, a1ps)
                else:
                    nc.vector.tensor_scalar_add(dst, a1ps, 0.0)

            # ---- W2 dense generation: w2sb[p, nh, m, c] ----
            # dg[k, m, c] = val2[k, m] * msk2[k, c] (expanded per m-block)
            dg = consts.tile([32, D, C], F16)
            w2sb = consts.tile([128, 2, D, C], F16)
            for mblk in range(D // MBLK):
                ms = slice(mblk * MBLK, (mblk + 1) * MBLK)
                nc.vector.scalar_tensor_tensor(
                    out=dg[:, ms, :],
                    in0=val2[:, ms].rearrange("p (m c) -> p m c", c=1)
                        .broadcast_to([32, MBLK, C]),
                    scalar=1.0,
                    in1=msk2.rearrange("p (m c) -> p m c", m=1)
                        .broadcast_to([32, MBLK, C]),
                    op0=AOP.mult, op1=AOP.mult)
                for nh in range(2):
                    gps = gpspool.tile([128, MBLK, C], F32)
                    nc.tensor.matmul(gps, cpb[:, nh * 128:(nh + 1) * 128],
                                     dg[:, ms, :], start=True, stop=True)
                    dst = w2sb[:, nh, ms, :]
                    if nh == 0:
                        nc.scalar.copy(dst, gps)
                    else:
                        nc.vector.tensor_scalar_add(dst, gps, 0.0)

            # ---- main pass: scores accumulate over all 512 chunks ----
            # the bias matmul opens the accumulation so it is off the
            # end-of-stream critical path
            sc = scpool.tile([BL, C], F32)
            nc.tensor.matmul(sc, ob[:, 0:BL], ob[:, BL:BL + C],
                             start=True, stop=False)
            for gi in range(len(sizes)):
                if gi in pre:
                    xt = pre[gi]
                else:
                    xt = xpool.tile([128, sizes[gi], BL], F16, tag="xt")
                    nc.sync.dma_start(
                        out=xt,
                        in_=xt_d[:, starts[gi]:starts[gi] + sizes[gi], :])
                for t in range(sizes[gi]):
                    q = starts[gi] + t
                    m, nh = q // 2, q % 2
                    last = (q == NQ - 1)
                    nc.tensor.matmul(sc, xt[:, t, :], w2sb[:, nh, m, :],
                                     start=False, stop=False)
                    nc.tensor.matmul(sc, xt[:, t, :], a1[:, m % 64, :],
                                     start=False, stop=last)

            # ---- softmax over the 16 free elements ----
            negmax = fin.tile([BL, 1], F32)
            nc.vector.tensor_reduce(out=negmax, in_=sc, axis=AXL.X,
                                    op=AOP.max, negate=True)
            e = fin.tile([BL, C], F32)
            sume = fin.tile([BL, 1], F32)
            nc.scalar.activation(out=e, in_=sc, func=AFT.Exp, bias=negmax,
                                 scale=1.0, accum_out=sume)
            rec = fin.tile([BL, 1], F32)
            nc.vector.reciprocal(rec, sume)
            nc.vector.tensor_scalar_mul(probs[:, 0, 0, :], e, rec)
            # prepared writeback: ctx indices are int32 zeros bitcast out of
            # cpa's zero padding, so the prep's only real deps are the cpa
            # load (early) and the probs RAW edge (relaxed post-scheduling;
            # descriptors only encode the probs ADDRESS -- the data is read
            # when the trigger fires, which is patched to wait for the final
            # DVE-lane count, i.e. the mul above). Desc-gen therefore runs
            # mid-stream on the idle Pool engine and the tail pays only
            # trigger+transfer+sem.
            ctxidx = cpa[:, 208:210].bitcast(mybir.dt.int32)
            dma_sem = nc.alloc_semaphore("probs_dma")
            nc.gpsimd.kv_writeback(out_d, probs, ctxidx,
                                   prepare_only=True, sem=dma_sem)
            nc.gpsimd.trigger_dma(count=None)

    _patch_writeback_sync(nc)
    nc.compile()
    # the end-of-program gather EventSemaphores only exist after compile;
    # BIR serialization for the device happens later (at run time), so these
    # post-compile patches reach both TimelineSim and hardware
    _hoist_dmasw_epilogue_wait(nc)
    _hoist_first_dma(nc)
    _cache["nc"] = nc
    return nc


def _hoist_first_dma(nc):
    """Move the first x-group DMA ahead of the SP queue's start-barrier wait.

    The instruction has no semaphore waits, its SBUF target is untouched by
    the preamble, and its consumers all wait on its DMAHW lane semaphore --
    the sync graph is completely unchanged, the DMA simply issues at t~0
    instead of after the ~640 ns all-engine start barrier, so the first
    transfer (and with it the entire gapless stream) starts that much
    earlier. SP arrives at the barrier ~650 ns later, which delays the
    other engines' release by ~120 ns; nothing on those queues is needed
    before the const tables land at ~8 us, so only the stream start moves.
    """
    fn = nc.m.functions[0]
    blocks = list(fn.blocks)
    b0, b1 = blocks[0].instructions, blocks[1].instructions
    idx = None
    for i, ins in enumerate(b1):
        if type(ins).__name__ == "InstDMACopy":
            idx = i
            break
    assert idx is not None
    first = b1[idx]
    si = first.sync_info
    assert not si.on_wait, [str(w) for w in si.on_wait]
    assert si.on_update and si.on_update[0].ant_name.startswith("DMAHW"), (
        [str(u) for u in si.on_update])
    b1.pop(idx)
    b0.insert(1, first)


def _patch_writeback_sync(nc):
    """Two post-scheduling fixes for the prepared output writeback.

    1. Point the prep's DMA-completion sem (OnUpdate[0], +16) at the DMASW0
       lane semaphore. Tile ticks the DMASW0 lane for a PREPARE_ONLY SWDGE
       prep and makes the epilogue wait on that lane sem, but bass bakes the
       user-passed `sem=` into the descriptor instead -- nothing would ever
       bump the lane sem (deadlock). This restores exactly the wiring Tile
       emits for gen_mode==0 SWDGE DMAs.
    2. Make the trigger wait on the DVE engine-lane sem reaching its final
       count -- the probs-producing divide is the last DVE instruction, so
       this is exactly "probs written" (a dedicated sem can't be used: DVE
       instructions have no spare sync-update slot, walrus rejects a
       then_inc there). The Pool engine-sem wait it replaces (desc-gen
       completion, done mid-stream) is satisfied tens of microseconds
       before the divide, so HW desc-gen-before-trigger ordering is
       preserved with huge margin.
    3. Relax the prep's RAW wait on the divide (a DVE engine-sem wait).
       The prep only encodes the probs ADDRESS into descriptors; the data
       is read when the trigger fires, which fix 2 orders after the
       divide. Leaving the wait in place would serialize the ~1us Q7
       desc-gen into the end-of-program tail.
    """
    prep = None
    trig = None
    dmasw = None
    dve_sem = None
    dve_total = 0

    def walk(blocks):
        nonlocal prep, trig, dmasw, dve_sem, dve_total
        for b in blocks:
            for ins in b.instructions:
                tn = type(ins).__name__
                si = getattr(ins, "sync_info", None)
                if tn == "InstKVWritebackAnt":
                    prep = ins
                elif tn == "InstTriggerDma":
                    trig = ins
                if si is not None:
                    for w in si.on_wait:
                        if w.ant_name and w.ant_name.startswith("DMASW"):
                            dmasw = w
                    for u in si.on_update:
                        if u.ant_name and u.ant_name.startswith("DVE"):
                            dve_sem = u
                            dve_total += 1
                for attr in ("blocks", "body"):
                    sub = getattr(ins, attr, None)
                    if sub:
                        walk(sub)

    walk(nc.m.functions[0].blocks)
    assert prep is not None and trig is not None, (prep, trig)
    assert dmasw is not None, dmasw
    assert dve_sem is not None and dve_total > 0, (dve_sem, dve_total)

    u0 = prep.sync_info.on_update[0]
    assert u0.ant_name == "probs_dma", u0
    u0.id = dmasw.id
    u0.ant_name = dmasw.ant_name

    w0 = trig.sync_info.on_wait[0]
    assert w0.ant_name.startswith("Pool"), w0
    w0.id = dve_sem.id
    w0.ant_name = dve_sem.ant_name
    w0.wait_value = dve_total

    relaxed = 0
    for w in prep.sync_info.on_wait:
        if w.ant_name and w.ant_name.startswith("DVE"):
            w.wait_value = 0
            relaxed += 1
    assert relaxed == 1, [str(w) for w in prep.sync_info.on_wait]


def _hoist_dmasw_epilogue_wait(nc):
    """Move the epilogue's DMASW0 wait into the LAST of the end-of-program
    gather EventSemaphores. These SP-queue instructions each carry a couple
    of lane-completion waits and run back-to-back; every wait must pass
    before the final drain, so their order is semantically free. The DMASW0
    wait (the output writeback, the last thing to finish) sits mid-sequence,
    so the instructions behind it run after the DMA sem instead of hiding
    inside its 900ns propagation. Swapping wait payloads fixes that.
    """
    seen_trigger = False
    gathers = []

    def walk(blocks):
        nonlocal seen_trigger
        for b in blocks:
            for ins in b.instructions:
                if type(ins).__name__ == "InstTriggerDma":
                    seen_trigger = True
                si = getattr(ins, "sync_info", None)
                if (seen_trigger and si and si.on_wait and not si.on_update
                        and type(ins).__name__ == "InstEventSemaphore"
                        and "barrier" not in ins.name):
                    gathers.append(ins)
                for attr in ("blocks", "body"):
                    sub = getattr(ins, attr, None)
                    if sub:
                        walk(sub)

    walk(nc.m.functions[0].blocks)
    holder = None
    hw = None
    for ins in gathers:
        for w in ins.sync_info.on_wait:
            if w.ant_name and w.ant_name.startswith("DMASW"):
                holder, hw = ins, w
    assert holder is not None, [i.name for i in gathers]
    last = gathers[-1]
    if last is not holder:
        lw = last.sync_info.on_wait[-1]
        hw.id, lw.id = lw.id, hw.id
        hw.ant_name, lw.ant_name = lw.ant_name, hw.ant_name
        hw.wait_value, lw.wait_value = lw.wait_value, hw.wait_value

    # Also hoist the SP engine Drain that follows the gathers but carries
    # no barrier update and a long-satisfied wait (Pool engine >=1, the
    # writeback prep at ~10us): it is a pure drain of the idle SP engine,
    # so running it before the DMASW wait is side-effect free and takes
    # its 25 ns out of the post-DMA critical path. The second Drain feeds
    # the exit-barrier gather and MUST stay after the DMASW wait (the
    # Pool-side sem cleanup would otherwise race the in-flight writeback).
    epi = nc.m.functions[0].blocks[2].instructions
    gather_names = {g.name for g in gathers}
    first_gather_idx = next(
        (i for i, ins in enumerate(epi) if ins.name in gather_names), None)
    for i, ins in enumerate(epi):
        if (type(ins).__name__ == "InstDrain"
                and str(getattr(ins, "engine", "")) == "EngineType.SP"
                and ins.sync_info and not ins.sync_info.on_update
                and first_gather_idx is not None and i > first_gather_idx):
            epi.pop(i)
            epi.insert(first_gather_idx, ins)
            break


def _host_prep(inputs, w1, w2, l, b):
    inputs = np.asarray(inputs, dtype=np.float32)
    w1 = np.asarray(w1, dtype=np.float32)
    w2 = np.asarray(w2, dtype=np.float32)
    l = np.asarray(l, dtype=np.float32)
    b = np.asarray(b, dtype=np.float32)

    g = np.float32(1.0 / (1.0 + np.exp(-np.float32(l[0]))))

    # A1 gen consts: st1[k=(c,r), p] = w1[c, p%64, r];
    # val1[k, j] = sign_r * g * w1[c, j, r]; msk1[k, c'] = (c' == k//8)
    w1t = w1.transpose(0, 2, 1)                                # [c, r, j]
    w1r = w1t.reshape(128, 64)
    st1 = np.concatenate([w1r, w1r], axis=1).astype(np.float16)
    signs = np.array([-1.0] * 4 + [1.0] * 4, np.float32)
    val1 = (w1t * g * signs[None, :, None]).reshape(128, 64)
    val1 = val1.astype(np.float16)
    msk1 = (np.arange(128)[:, None] // 8 ==
            np.arange(C)[None, :]).astype(np.float16)

    # W2 gen consts: st2[k, n] = (u2|v2)[k, n]; msk2[k, c'] = (c' == k%16);
    # scale[k] = +-(1-g) (val2 = scale * st2 is computed on device)
    u2, v2 = w2[:, :, 1], w2[:, :, 0]                          # [16, 256]
    st2 = np.concatenate([u2, v2], axis=0).astype(np.float16)  # [32, 256]
    msk2 = (np.arange(32)[:, None] % 16 ==
            np.arange(C)[None, :]).astype(np.float16)

    cpa = np.zeros((128, 256), np.float16)
    cpa[:, 0:128] = st1
    cpa[:, 128:192] = val1
    cpa[:, 192:208] = msk1
    cpb = np.zeros((32, 274), np.float16)
    cpb[:, 0:256] = st2
    cpb[:, 256:272] = msk2
    scale = np.zeros((32, 1), np.float32)
    scale[0:16] = 1.0 - g
    scale[16:32] = -(1.0 - g)
    cpb[:, 272:274] = scale.view(np.float16)
    cpc = np.zeros((1, 144), np.float16)
    cpc[0, 0:128] = 1.0
    cpc[0, 128:144] = b.astype(np.float16)

    # x: [1024, 256, 256] -> per-core xt[p, q, s] = x[s, q*128 + p]
    xt_all = inputs.astype(np.float16).reshape(N_CORES, BL, NQ, 128)
    xt_all = np.ascontiguousarray(xt_all.transpose(0, 3, 2, 1))

    shared = dict(cpa=cpa, cpb=cpb, cpc=cpc)
    in_maps = []
    for core in range(N_CORES):
        m = dict(shared)
        m["xt"] = xt_all[core]
        in_maps.append(m)
    return in_maps


def kernel(inputs, w1, w2, l, b, _trace=False):
    nc = _build()
    in_maps = _host_prep(inputs, w1, w2, l, b)
    res = run_bass_kernel_spmd(nc, in_maps, core_ids=list(range(N_CORES)),
                               trace=_trace)
    out = np.concatenate([r["probs"].reshape(BL, C) for r in res.results],
                         axis=0)
    if _trace:
        kernel.last_results = res
    return out

